# revision 1
# baseline (speedup 1.0000x reference)
"""Trainium2 Bass kernel for nn_ModelR_37022618091886.

Model: y = MLP(x) with 5 layers (leaky-relu 0.01 between), then per-example
triangular scatter of the 2080 outputs into an upper-triangular 64x64 matrix
(diagonal entries abs'ed), output shape (64, 64, 8192).

Strategy:
  - Data-parallel over batch across 8 cores (1024 examples/core), weights
    replicated.
  - Activations are kept feature-major on chip: h^T [features, batch], so the
    contraction dim of every matmul is already on SBUF partitions and the
    final layer directly produces y^T [2080, batch] = (almost) the output.
  - W3's columns are permuted on the host so that the final layer's output
    rows land in output-row order: row block i holds (i,j) for j=i..63
    ascending.  The "scatter" then degenerates to ~80 contiguous-run DMAs.
  - The diagonal abs is y = max(y, c*y) with c = -1 on diagonal rows else +1
    (per-partition scalar), fused after the bias add.
  - Strict-lower-triangle rows of the output are never written; the runtime
    pre-zeroes/donates zeroed output buffers (see run_bass_via_pjrt), so they
    read back as exact zeros.
  - Matmuls run as float32r (TRN2 reduced-precision fp32 mode, full rate on
    the PE array vs 4 cycles/row for strict fp32).
"""

import os

import numpy as np

CPV = 64
L = CPV * (CPV + 1) // 2  # 2080
LT = 17  # number of 128-row tiles covering L (padded)
LP = LT * 128  # 2176
D_IN = 1024
H = 2048
B = 8192
N_CORES = 8
BC = B // N_CORES  # 1024 batch per core
NCH = 2  # moving-dim chunks of 512 (fp32 PSUM bank limit)
NSZ = BC // NCH

_DTYPE = os.environ.get("KERNEL_DTYPE", "f32r")  # "f32r" | "f32"
_REPS = int(os.environ.get("KERNEL_REPS", "1"))  # timing aid: unroll kernel R times

_compiled_nc = None


def _offsets():
    off = np.zeros(CPV + 1, dtype=np.int64)
    for i in range(CPV):
        off[i + 1] = off[i] + (CPV - i)
    return off


def _perm_and_coefs():
    """Column permutation for W3 + diag coefficient vector.

    New output order m: for i in 0..63, for j in i..63 -> m = off[i] + (j-i).
    Torch/ref order k: for i, for idx_y: col j = 63-idx_y -> k = off[i] + (63-j).
    """
    off = _offsets()
    perm = np.empty(L, dtype=np.int64)
    for i in range(CPV):
        n = CPV - i
        perm[off[i] : off[i] + n] = off[i] + np.arange(n)[::-1]
    coef = np.ones(LP, dtype=np.float32)
    coef[off[:CPV]] = -1.0  # diagonal (i,i) sits at the start of block i
    return perm, coef


def _out_runs():
    """Maximal runs of consecutive final-layer rows that map to consecutive
    output rows: list of (tile, p0, length, out_row0)."""
    off = _offsets()
    runs = []
    for t in range(LT):
        m0t, m1t = t * 128, min((t + 1) * 128, L)
        m = m0t
        while m < m1t:
            i = int(np.searchsorted(off, m, side="right") - 1)
            end = int(min(m1t, off[i] + (CPV - i)))
            runs.append((t, m - m0t, end - m, 65 * i + (m - off[i])))
            m = end
    assert sum(r[2] for r in runs) == L
    return runs


def _build(wp_bufs=4, ps_bufs=8, yp_bufs=2, reps=None):
    import concourse.bacc as bacc
    import concourse.mybir as mybir
    import concourse.tile as tile

    F32 = mybir.dt.float32
    MMDT = mybir.dt.float32r if _DTYPE == "f32r" else mybir.dt.float32
    ACT = mybir.ActivationFunctionType

    nc = bacc.Bacc("TRN2", target_bir_lowering=False, debug=False, num_devices=N_CORES)

    xt = nc.dram_tensor("xt", [D_IN, BC], MMDT, kind="ExternalInput")
    w1 = nc.dram_tensor("w1", [D_IN, H], MMDT, kind="ExternalInput")
    w2 = nc.dram_tensor("w2", [H, H], MMDT, kind="ExternalInput")
    w21 = nc.dram_tensor("w21", [H, H], MMDT, kind="ExternalInput")
    w22 = nc.dram_tensor("w22", [H, H], MMDT, kind="ExternalInput")
    w3 = nc.dram_tensor("w3", [H, LP], MMDT, kind="ExternalInput")
    b1t = nc.dram_tensor("b1t", [128, H // 128], F32, kind="ExternalInput")
    b2t = nc.dram_tensor("b2t", [128, H // 128], F32, kind="ExternalInput")
    b21t = nc.dram_tensor("b21t", [128, H // 128], F32, kind="ExternalInput")
    b22t = nc.dram_tensor("b22t", [128, H // 128], F32, kind="ExternalInput")
    b3t = nc.dram_tensor("b3t", [128, LT], F32, kind="ExternalInput")
    c3t = nc.dram_tensor("c3t", [128, LT], F32, kind="ExternalInput")
    out = nc.dram_tensor("out", [CPV * CPV, BC], F32, kind="ExternalOutput")

    runs = _out_runs()

    with tile.TileContext(nc) as tc:
        with (
            tc.tile_pool(name="acts", bufs=1) as acts,
            tc.tile_pool(name="wp", bufs=wp_bufs) as wp,
            tc.tile_pool(name="cst", bufs=1) as cst,
            tc.tile_pool(name="yp", bufs=yp_bufs) as yp,
            tc.tile_pool(name="ycp", bufs=1) as ycp,
            tc.tile_pool(name="ps", bufs=ps_bufs, space="PSUM") as ps,
        ):
            # --- constants (biases / diag coefs) ---
            bias_tiles = {}
            for name, dram, nt in (
                ("b1", b1t, 16),
                ("b2", b2t, 16),
                ("b21", b21t, 16),
                ("b22", b22t, 16),
                ("b3", b3t, LT),
                ("c3", c3t, LT),
            ):
                t = cst.tile([128, nt], F32, tag=name, name=name)
                nc.sync.dma_start(t[:], dram[:, :])
                bias_tiles[name] = t

            for _rep in range(_REPS if reps is None else reps):
              # --- x^T load: 8 k-tiles, split in halves to spread DMA queues ---
              x_tiles = []
              for k in range(D_IN // 128):
                t = acts.tile([128, BC], MMDT, tag=f"x{k}", name=f"x{k}")
                nc.scalar.dma_start(t[:, :NSZ], xt[k * 128 : (k + 1) * 128, :NSZ])
                nc.scalar.dma_start(t[:, NSZ:], xt[k * 128 : (k + 1) * 128, NSZ:])
                x_tiles.append(t)

              def load_weight_block(wt, w_dram, kt, m, lname):
                  """Load W[:, m*128:(m+1)*128] as kt [128,128] k-tiles into wt,
                  batched 4 k-tiles per DMA, alternating HWDGE issue engines."""
                  for ci, k0 in enumerate(range(0, kt, 4)):
                      nk = min(4, kt - k0)
                      eng = nc.sync if ci % 2 == 0 else nc.scalar
                      eng.dma_start(
                          wt[:, k0 * 128 : (k0 + nk) * 128].rearrange(
                              "p (k c) -> p k c", k=nk
                          ),
                          w_dram[
                              k0 * 128 : (k0 + nk) * 128, m * 128 : (m + 1) * 128
                          ].rearrange("(k p) c -> p k c", p=128),
                      )

              def layer(lname, w_dram, kt, mt, h_in, btile, out_tag_prefix):
                  """One hidden layer: h_out[m] = lrelu(W[:,m]^T @ h_in + b[m])."""
                  h_out = []
                  for m in range(mt):
                      wt = wp.tile([128, kt * 128], MMDT, tag="w", name=f"w_{lname}_{m}")
                      load_weight_block(wt, w_dram, kt, m, lname)
                      ps0 = ps.tile([128, NSZ], F32, tag="ps", name=f"ps0_{lname}_{m}")
                      ps1 = ps.tile([128, NSZ], F32, tag="ps", name=f"ps1_{lname}_{m}")
                      for k in range(kt):
                          lhsT = wt[:, k * 128 : (k + 1) * 128]
                          nc.tensor.matmul(
                              ps0[:],
                              lhsT,
                              h_in[k][:, :NSZ],
                              start=(k == 0),
                              stop=(k == kt - 1),
                          )
                          nc.tensor.matmul(
                              ps1[:],
                              lhsT,
                              h_in[k][:, NSZ:],
                              start=(k == 0),
                              stop=(k == kt - 1),
                          )
                      ht = acts.tile(
                          [128, BC],
                          MMDT,
                          tag=f"{out_tag_prefix}{m}",
                          name=f"h_{lname}_{m}",
                      )
                      nc.scalar.activation(
                          ht[:, :NSZ],
                          ps0[:],
                          ACT.Lrelu,
                          bias=btile[:, m : m + 1],
                          scale=1.0,
                          alpha=0.01,
                      )
                      nc.scalar.activation(
                          ht[:, NSZ:],
                          ps1[:],
                          ACT.Lrelu,
                          bias=btile[:, m : m + 1],
                          scale=1.0,
                          alpha=0.01,
                      )
                      h_out.append(ht)
                  return h_out

              h1 = layer("l1", w1, D_IN // 128, H // 128, x_tiles, bias_tiles["b1"], "a")
              h2 = layer("l2", w2, H // 128, H // 128, h1, bias_tiles["b2"], "b")
              h3 = layer("l3", w21, H // 128, H // 128, h2, bias_tiles["b21"], "a")
              h4 = layer("l4", w22, H // 128, H // 128, h3, bias_tiles["b22"], "b")

              # --- final layer + scatter ---
              runs_by_tile = {}
              for r in runs:
                  runs_by_tile.setdefault(r[0], []).append(r)
              kt = H // 128
              for m in range(LT):
                  wt = wp.tile([128, kt * 128], MMDT, tag="w", name=f"w_l5_{m}")
                  load_weight_block(wt, w3, kt, m, "l5")
                  ps0 = ps.tile([128, NSZ], F32, tag="ps", name=f"ps0_l5_{m}")
                  ps1 = ps.tile([128, NSZ], F32, tag="ps", name=f"ps1_l5_{m}")
                  for k in range(kt):
                      lhsT = wt[:, k * 128 : (k + 1) * 128]
                      nc.tensor.matmul(
                          ps0[:], lhsT, h4[k][:, :NSZ], start=(k == 0), stop=(k == kt - 1)
                      )
                      nc.tensor.matmul(
                          ps1[:], lhsT, h4[k][:, NSZ:], start=(k == 0), stop=(k == kt - 1)
                      )
                  y = yp.tile([128, BC], F32, tag="y", name=f"y_{m}")
                  nc.scalar.activation(
                      y[:, :NSZ],
                      ps0[:],
                      ACT.Identity,
                      bias=bias_tiles["b3"][:, m : m + 1],
                  )
                  nc.scalar.activation(
                      y[:, NSZ:],
                      ps1[:],
                      ACT.Identity,
                      bias=bias_tiles["b3"][:, m : m + 1],
                  )
                  yc = ycp.tile([128, BC], F32, tag="yc", name=f"yc_{m}")
                  nc.vector.tensor_scalar_mul(yc[:], y[:], bias_tiles["c3"][:, m : m + 1])
                  nc.vector.tensor_max(y[:], y[:], yc[:])
                  for _, p0, ln, r0 in runs_by_tile.get(m, []):
                      nc.sync.dma_start(out[r0 : r0 + ln, :], y[p0 : p0 + ln, :])

    nc.compile()
    return nc


def _get_nc():
    global _compiled_nc
    if _compiled_nc is None:
        _compiled_nc = _build()
    return _compiled_nc


def _tile_weight(W):
    """[K, M] -> [MT*128, KT*128] host pre-tiling (see _build docstring)."""
    K, M = W.shape
    kt, mt = K // 128, M // 128
    return np.ascontiguousarray(
        W.reshape(kt, 128, mt, 128).transpose(2, 1, 0, 3).reshape(mt * 128, kt * 128)
    )


def prepare_in_maps(x, W1, b1, W2, b2, W21, b21, W22, b22, W3, b3):
    x = np.asarray(x, dtype=np.float32)
    perm, coef = _perm_and_coefs()
    w3p = np.zeros((H, LP), dtype=np.float32)
    w3p[:, :L] = np.asarray(W3, np.float32)[:, perm]
    b3p = np.zeros(LP, dtype=np.float32)
    b3p[:L] = np.asarray(b3, np.float32)[perm]

    def tile_bias(b, nt):
        return np.ascontiguousarray(np.asarray(b, np.float32).reshape(nt, 128).T)

    common = {
        "w1": np.asarray(W1, np.float32),
        "w2": np.asarray(W2, np.float32),
        "w21": np.asarray(W21, np.float32),
        "w22": np.asarray(W22, np.float32),
        "w3": w3p,
        "b1t": tile_bias(b1, 16),
        "b2t": tile_bias(b2, 16),
        "b21t": tile_bias(b21, 16),
        "b22t": tile_bias(b22, 16),
        "b3t": tile_bias(b3p, LT),
        "c3t": tile_bias(coef, LT),
    }
    xT = np.ascontiguousarray(x.T)  # [D_IN, B]
    return [
        {**common, "xt": np.ascontiguousarray(xT[:, c * BC : (c + 1) * BC])}
        for c in range(N_CORES)
    ]


def kernel(
    x, W1, b1, W2, b2, W21, b21, W22, b22, W3, b3
):  # noqa: N803 - match reference names
    nc = _get_nc()
    in_maps = prepare_in_maps(x, W1, b1, W2, b2, W21, b21, W22, b22, W3, b3)

    from concourse.bass_utils import run_bass_kernel_spmd

    res = run_bass_kernel_spmd(nc, in_maps, core_ids=list(range(N_CORES)))
    return np.concatenate(
        [res.results[c]["out"].reshape(CPV, CPV, BC) for c in range(N_CORES)], axis=2
    )

